# revision 74
# baseline (speedup 1.0000x reference)
"""Causal self-attention TRN2 kernel (B=2, L=2048, D=1024, H=16, dh=64).

Sharding: 8 cores = 2 batches x 4 head-groups. Core c handles batch c//4 and
heads [4g..4g+3] where g = c % 4, as two head-pairs ("units").

Per-core device program (SPMD, same program all cores, different data),
everything fp16 on the PE (full rate) with fp32 PSUM accumulation:
  phase 1 (per unit): qT/kT = W.T @ x (head dims on partitions); V computed
      directly in [kpos, dh] orientation with a fused ones column (v65) so
      the AV matmul also produces softmax row-sums.
  phase 2 (per unit, per 512-wide q-block): S^T chunks [128k x 512q] on PE,
      exp on ScalarE (PSUM->SBUF fp16, scale=1/8, no max-sub), causal
      triangle mask via one fp16 DVE multiply per diagonal chunk, then
      flipped AV: the P^T chunk is the stationary operand and v65 the
      moving one -> [128q, 64dh | l] PSUM tiles. Normalize with
      per-partition scalars (DVE reciprocal + tensor_scalar_mul),
      PE-transpose back into outT. AV consumption runs DEPTH chunks behind
      S production so the PE never waits on the ScalarE exp; each block's
      normalize/transpose runs as the next block's prelude.
  phase 3: y[q,:] partial = outT_u0/u1 x w_out, fp32 PSUM, fp16 DMA out.
Host: y[b] = sum of the 4 partial outputs for batch b.
PE work is interleaved across phases (qkv(u1) inside the attn(u0) window,
outproj inside the attn(u1) window) to keep the tensor engine saturated.
"""

import numpy as np

import concourse.bass as bass
import concourse.mybir as mybir
from concourse import bacc
import concourse.tile as tile
from concourse.bass_utils import run_bass_kernel_spmd

F32 = mybir.dt.float32
FP16 = mybir.dt.float16
EXP = mybir.ActivationFunctionType.Exp
COPY = mybir.ActivationFunctionType.Copy

B, L, D = 2, 2048, 1024
H, DH = 16, 64
NCORES = 8
NQB = L // 512          # q-blocks per sequence (4)
DEPTH = 5               # chunks of S/exp lookahead before AV consumes

_CACHE = {}
LAST_RESULT = None      # BassKernelResults of the most recent run (for test.py)
DEBUG_TAPS = False      # adds qT/kT/v65/outT dram outputs for debug.py


def _build():
    nc = bacc.Bacc("TRN2", target_bir_lowering=False, debug=False,
                   num_devices=NCORES)

    xt_d = nc.dram_tensor("xt", [D, L], FP16, kind="ExternalInput").ap()
    # wq pre-packed on host to the SBUF layout: [128, 6 sections, 8 d, 128]
    wq_d = nc.dram_tensor("wq", [128, 6 * 8 * 128], FP16,
                          kind="ExternalInput").ap()
    wout_d = nc.dram_tensor("wout", [2, 128, 1024], FP16,
                            kind="ExternalInput").ap()
    consts_d = nc.dram_tensor("consts", [128, 416], FP16,
                              kind="ExternalInput").ap()
    y_d = nc.dram_tensor("y", [L, D], FP16, kind="ExternalOutput").ap()

    with tile.TileContext(nc) as tc:
        with tc.tile_pool(name="persist", bufs=1) as pp, \
             tc.tile_pool(name="work", bufs=1) as wp, \
             tc.tile_pool(name="psstage", bufs=3, space="PSUM") as ps_stage, \
             tc.tile_pool(name="psav", bufs=2, space="PSUM") as ps_av:

            # ---- input DMAs (ordered so early chains' deps land first) ----
            # wq SBUF layout is section-major: section s = (u*3 + ci), cols
            # [s*1024 + d*128 + c]; each section is one contiguous 728ns DMA.
            wq = pp.tile([128, 6 * 1024], FP16, name="wq")
            xt = pp.tile([128, 8 * L], FP16, name="xt")
            xt_dst = xt.rearrange("p (d l) -> p d l", d=8)
            xt_src = xt_d.rearrange("(a b) c -> b a c", b=128)

            def load_wq(u, ci):
                s = u * 3 + ci
                nc.sync.dma_start(out=wq[:, s * 1024:(s + 1) * 1024],
                                  in_=wq_d[:, s * 1024:(s + 1) * 1024])

            def load_xt(q, half=None, eng=None):
                cs = slice(q * 512, (q + 1) * 512) if half is None else \
                    slice(q * 512 + half * 256, q * 512 + half * 256 + 256)
                (eng or nc.sync).dma_start(out=xt_dst[:, :, cs],
                                           in_=xt_src[:, :, cs])

            load_wq(0, 0)
            load_xt(0, 0)
            load_xt(0, 1)
            load_wq(0, 1)
            load_wq(0, 2)
            consts = pp.tile([128, 416], FP16, name="consts")
            nc.sync.dma_start(out=consts, in_=consts_d)
            ident = consts[:, 0:128]      # eye(128)
            tri2 = consts[:, 128:384].rearrange("p (h q) -> p h q", h=2)
            ones32 = consts[:, 384:416]
            for ci in range(3):
                load_wq(1, ci)
            load_xt(1)
            load_xt(2)
            load_xt(3)
            wout = pp.tile([128, 2048], FP16, name="wout")
            nc.sync.dma_start(out=wout,
                              in_=wout_d.rearrange("a b c -> b a c"))

            # persistent per-unit tensors
            qT = [pp.tile([128, L], FP16, name=f"qT{u}") for u in range(2)]
            kT = [pp.tile([128, L], FP16, name=f"kT{u}") for u in range(2)]
            # V with ones column: per head 16 chunks x (64 dh + 1)
            v65 = [pp.tile([128, 2 * 16 * 65], FP16, name=f"v65{u}")
                   for u in range(2)]
            outT = [pp.tile([128, L], FP16, name=f"outT{u}") for u in range(2)]

            def copy_op(act, out, in_):
                # Act only while it is otherwise idle (window A); fillers
                # inside the attn windows copy on DVE to keep exp flowing.
                if act:
                    nc.scalar.activation(out, in_, COPY)
                else:
                    nc.vector.tensor_copy(out, in_)

            def emit_qk_half(u, ci, r, w=512, act=False):
                """One 512-col half: 8 accumulating matmuls + one copy."""
                dest = qT[u] if ci == 0 else kT[u]
                so = (u * 3 + ci) * 1024
                acc = ps_stage.tile([128, 512], F32,
                                    name=f"qk{u}{ci}{r}", tag="stage")
                for piece in range(512 // w):
                    c0 = r * 512 + piece * w
                    for d in range(8):
                        nc.tensor.matmul(
                            acc[:, piece * w:(piece + 1) * w],
                            wq[:, so + d * 128:so + (d + 1) * 128],
                            xt[:, d * L + c0:d * L + c0 + w],
                            start=(d == 0), stop=(d == 7),
                            skip_group_check=True)
                copy_op(act, dest[:, r * 512:(r + 1) * 512], acc)

            def emit_v_chain(u, g, act=False):
                # 4 k-chunks of 128; V lands in [kpos, dh] orientation
                so = (u * 3 + 2) * 1024
                acc = ps_stage.tile([128, 512], F32, name=f"v{u}{g}",
                                    tag="stage")
                for jc in range(4):
                    j = g * 4 + jc
                    for d in range(8):
                        nc.tensor.matmul(
                            acc[:, jc * 128:(jc + 1) * 128],
                            xt[:, d * L + j * 128:d * L + (j + 1) * 128],
                            wq[:, so + d * 128:so + (d + 1) * 128],
                            start=(d == 0), stop=(d == 7),
                            skip_group_check=True)
                accv = acc.rearrange("p (c v) -> p c v", c=4)
                v65v = v65[u].rearrange("p (c v) -> p c v", c=32)
                for h in range(2):
                    copy_op(act, v65v[:, h * 16 + g * 4:h * 16 + (g + 1) * 4,
                                      0:64],
                            accv[:, :, h * 64:(h + 1) * 64])

            def qkv_chains(u, act3=False):
                # half-granularity (~1.7us each) so fillers never starve Act;
                # ordered so the first chains only need xt q-blocks 0/1
                w0 = 256 if u == 0 else 512
                yield lambda: emit_qk_half(u, 0, 0, w=w0, act=act3)
                yield lambda: emit_qk_half(u, 0, 1, w=w0, act=act3)
                yield lambda: emit_qk_half(u, 1, 0, act=act3)
                yield lambda: emit_qk_half(u, 1, 1, act=act3)
                yield lambda: emit_v_chain(u, 0)
                yield lambda: emit_v_chain(u, 1)
                yield lambda: emit_qk_half(u, 0, 2)
                yield lambda: emit_qk_half(u, 0, 3)
                yield lambda: emit_qk_half(u, 1, 2)
                yield lambda: emit_qk_half(u, 1, 3)
                yield lambda: emit_v_chain(u, 2)
                yield lambda: emit_v_chain(u, 3)

            def finalize_norm(u, I, h, av):
                """DVE part: 1/l and normalize the 4 q-tiles of head h."""
                avv = av.rearrange("p (t c) -> p t c", t=4)
                rt = wp.tile([128, 4], F32, name=f"rt{u}{I}{h}",
                             tag="rt", bufs=4)
                rtv = rt.rearrange("p (t c) -> p t c", c=1)
                nc.vector.reciprocal(rtv, avv[:, :, 64:65])
                osb = wp.tile([128, 256], FP16, name=f"osb{u}{I}{h}",
                              tag="osb", bufs=4)
                osbv = osb.rearrange("p (t c) -> p t c", t=4)
                nc.vector.tensor_mul(osbv, avv[:, :, 0:64],
                                     rtv.broadcast_to([128, 4, 64]))
                return osb

            def finalize_tp(u, I, h, osb):
                """PE part: transpose normalized tiles into outT."""
                tp = ps_stage.tile([128, 512], FP16, name=f"tp{u}{I}{h}",
                                   tag="stage")
                for t in range(4):
                    nc.tensor.transpose(tp[0:64, t * 128:(t + 1) * 128],
                                        osb[:, t * 64:(t + 1) * 64], ident)
                nc.vector.tensor_copy(
                    outT[u][h * 64:(h + 1) * 64, I * 512:(I + 1) * 512],
                    tp[0:64, :])

            def finalize_qt_norm(u, I, h, t, av):
                """Tail variant, DVE part for a single q-tile."""
                avv = av.rearrange("p (t c) -> p t c", t=4)
                rt = wp.tile([128, 4], F32, name=f"rt{u}{I}{h}{t}",
                             tag="rt", bufs=4)
                nc.vector.reciprocal(rt[:, 0:1], avv[:, t:t + 1, 64:65])
                osb = wp.tile([128, 256], FP16, name=f"ot{u}{I}{h}{t}",
                              tag="osb", bufs=4)
                nc.vector.tensor_scalar_mul(osb[:, 0:64],
                                            avv[:, t:t + 1, 0:64],
                                            rt[:, 0:1])
                return osb

            def finalize_qt_tp(u, I, h, t, osb):
                """Tail variant, PE part (stage-tag psum: av stays live)."""
                tp = ps_stage.tile([128, 128], FP16, name=f"tq{u}{I}{h}{t}",
                                   tag="stage")
                nc.tensor.transpose(tp[0:64, :], osb[:, 0:64], ident)
                nc.vector.tensor_copy(
                    outT[u][h * 64:(h + 1) * 64,
                            (4 * I + t) * 128:(4 * I + t + 1) * 128],
                    tp[0:64, :])

            def outproj_qc(qc, act_copy=False, split=False):
                qs = slice(qc * 128, (qc + 1) * 128)
                ysb = wp.tile([128, 1024], FP16, name=f"ys{qc}",
                              tag="ysb", bufs=4)
                for nck in range(2):
                    ns = slice(nck * 512, (nck + 1) * 512)
                    yps = ps_stage.tile([128, 512], F32,
                                        name=f"y{qc}{nck}", tag="stage")
                    nc.tensor.matmul(yps, outT[0][:, qs], wout[:, ns],
                                     start=True, stop=False,
                                     skip_group_check=True)
                    nc.tensor.matmul(yps, outT[1][:, qs],
                                     wout[:, 1024 + ns.start:1024 + ns.stop],
                                     start=False, stop=True,
                                     skip_group_check=True)
                    if act_copy and nck == 0:
                        nc.scalar.activation(ysb[:, ns], yps, COPY)
                    else:
                        nc.vector.tensor_copy(ysb[:, ns], yps)
                    if split:   # last q-tile: stream each half immediately
                        nc.sync.dma_start(out=y_d[qs, ns], in_=ysb[:, ns])
                if not split:
                    nc.sync.dma_start(out=y_d[qs, :], in_=ysb)

            def attn_block(u, I, prelude, chainq, oprojq, fill_phase,
                           fill_every, fill_max, tail=False, depth=DEPTH):
                nj = 4 * (I + 1)
                av = [None, None]
                pts = []
                pops = 0
                opops = 0
                osbs = {}        # tail: (h, t) -> osb awaiting transpose
                for j in range(nj + depth + (3 if tail else 0)):
                    if j < nj:
                        m = j - 4 * I
                        qlo = max(m, 0) * 128
                        st = ps_stage.tile([128, 1024], F32,
                                           name=f"st{u}{I}{j}", tag="stage")
                        for h in range(2):
                            nc.tensor.matmul(
                                st[:, h * 512 + qlo:(h + 1) * 512],
                                kT[u][h * 64:(h + 1) * 64,
                                      j * 128:(j + 1) * 128],
                                qT[u][h * 64:(h + 1) * 64,
                                      I * 512 + qlo:(I + 1) * 512],
                                start=True, stop=True, skip_group_check=True)
                        pt = wp.tile([128, 1024], FP16, name=f"pt{u}{I}{j}",
                                     tag="pt", bufs=9)
                        stv = st.rearrange("p (h q) -> p h q", h=2)
                        ptv = pt.rearrange("p (h q) -> p h q", h=2)
                        nc.scalar.activation(ptv[:, :, qlo:512],
                                             stv[:, :, qlo:512], EXP,
                                             scale=0.125)
                        pts.append(pt)
                    # masks trail S/exp by 2 chunks so DVE never queues on Act
                    mc = j - 2
                    if 0 <= mc < nj and mc >= 4 * I:
                        mqlo = (mc - 4 * I) * 128
                        pmv = pts[mc].rearrange("p (h q) -> p h q", h=2)
                        nc.gpsimd.tensor_mul(pmv[:, :, mqlo:mqlo + 128],
                                             pmv[:, :, mqlo:mqlo + 128],
                                             tri2)
                    if j == 2:
                        # av groups share one psum bank, so never start=True
                        # (start lazily zeroes the whole 2KB region, wiping
                        # sibling q-tile accumulators): memset instead.
                        for h in range(2):
                            av[h] = ps_av.tile([128, 260], F32,
                                               name=f"av{u}{I}{h}", tag="av")
                            nc.vector.memset(av[h], 0.0)
                    if j == 3:
                        for f in prelude:      # prev block transposes (PE)
                            f()
                    if j >= fill_phase and (j - fill_phase) % fill_every == 0:
                        if chainq and pops < fill_max[0]:
                            chainq.pop(0)()
                            pops += 1
                        elif oprojq and opops < fill_max[1]:
                            oprojq.pop(0)()
                            opops += 1
                    ja = j - depth
                    if 0 <= ja < nj:
                        ma = ja - 4 * I
                        pta = pts[ja]
                        for t in range(max(ma, 0), 4):
                            qt = 4 * I + t
                            for h in range(2):
                                nc.tensor.matmul(
                                    av[h][:, t * 65:t * 65 + 65],
                                    pta[:, h * 512 + t * 128:
                                        h * 512 + (t + 1) * 128],
                                    v65[u][:, h * 1040 + ja * 65:
                                           h * 1040 + (ja + 1) * 65],
                                    start=False, stop=(ja == qt),
                                    skip_group_check=True)
                        if tail and ma >= 0:
                            for h in range(2):
                                osbs[(h, ma)] = finalize_qt_norm(
                                    u, I, h, ma, av[h])
                    if tail:
                        mb = j - depth - 1 - 4 * I
                        if mb >= 0 and (0, mb) in osbs:
                            for h in range(2):
                                finalize_qt_tp(u, I, h, mb,
                                               osbs.pop((h, mb)))
                        m2 = j - depth - 2 - 4 * I
                        if 0 <= m2 <= 3:
                            outproj_qc(4 * I + m2, act_copy=True,
                                       split=(m2 == 3))
                if tail:
                    return []
                # norms (DVE) run now, right after the final AV stops; the
                # PE transposes become the next block's prelude.
                osbf = [finalize_norm(u, I, h, av[h]) for h in range(2)]
                return [lambda h=h, o=osbf[h]: finalize_tp(u, I, h, o)
                        for h in range(2)]

            # ---- emission schedule ----
            # warm the Act exp table while the PE is still loading inputs
            junk = wp.tile([128, 1], FP16, name="junk", tag="junk", bufs=1)
            nc.vector.memset(junk, 0.0)
            nc.scalar.activation(junk, junk, EXP, scale=0.0)
            for u in range(2):
                nc.vector.tensor_copy(
                    v65[u].rearrange("p (c v) -> p c v", c=32)[:, :, 64:65],
                    ones32)
            c0 = list(qkv_chains(0))
            for ch in [c0[0], c0[2], c0[4]]:  # Qh0, Kh0, Vg0: block 0's deps
                ch()
            c1 = list(qkv_chains(1))
            # u0/u1 blocks alternate; two filler queues ordered by first use
            # (and by xt q-block arrival) so outproj lands in the exp-heavy
            # late blocks.
            chainq = [c1[0], c1[2], c1[4],          # u1 Qh0, Kh0, Vg0 (xt0)
                      c0[1], c0[3],                 # u0 Qh1, Kh1 (xt1)
                      c1[1], c1[3],                 # u1 Qh1, Kh1
                      c0[5], c1[5],                 # Vg1 u0, u1
                      c0[6], c0[8],                 # u0 Qh2, Kh2 (xt2)
                      c0[7], c0[9],                 # u0 Qh3, Kh3 (xt3)
                      c1[6], c1[7],                 # u1 Qh2, Qh3
                      c0[10],                       # u0 Vg2
                      c1[8], c1[9],                 # u1 Kh2, Kh3
                      c1[10],                       # u1 Vg2
                      c0[11], c1[11]]               # Vg3 u0, u1
            oprojq = []
            prel = []
            sched = [  # (u, I, phase, every, (chain_max, oproj_max), depth)
                (0, 0, 2, 1, (7, 0), 5), (1, 0, 3, 2, (3, 0), 5),
                (0, 1, 2, 2, (3, 0), 5), (1, 1, 4, 2, (3, 1), 5),
                (0, 2, 2, 2, (2, 1), 5), (1, 2, 4, 2, (1, 2), 5),
                (0, 3, 4, 2, (2, 3), 5), (1, 3, 4, 2, (0, 99), 5)]
            for u, I, phase, every, fmax, dep in sched:
                tail = (u == 1 and I == NQB - 1)
                prel = attn_block(u, I, prel, chainq, oprojq,
                                  fill_phase=phase, fill_every=every,
                                  fill_max=fmax, tail=tail, depth=dep)
                if u == 1 and I < NQB - 1:
                    oprojq += [lambda qc=qc: outproj_qc(qc)
                               for qc in range(4 * I, 4 * (I + 1))]
            while chainq:
                chainq.pop(0)()
            while oprojq:
                oprojq.pop(0)()

            if DEBUG_TAPS:
                taps = {"qT0": qT[0], "kT0": kT[0], "outT0": outT[0],
                        "v650": v65[0]}
                for nm, t_sb in taps.items():
                    td = nc.dram_tensor(nm, list(t_sb.shape), FP16,
                                        kind="ExternalOutput").ap()
                    nc.sync.dma_start(out=td, in_=t_sb)

    nc.compile()
    return nc


def _host_inputs(x, w_qkv, w_out):
    """Build per-core input maps."""
    x = np.asarray(x, dtype=np.float32)
    w_qkv = np.asarray(w_qkv, dtype=np.float32)
    w_out = np.asarray(w_out, dtype=np.float32)

    xts = [np.ascontiguousarray(x[b].T).astype(np.float16) for b in range(B)]

    consts = np.zeros((128, 416), dtype=np.float16)
    consts[:, 0:128] = np.eye(128)
    kk = np.arange(128)[:, None]
    qq = np.arange(128)[None, :]
    tri = (kk <= qq).astype(np.float16)
    consts[:, 128:256] = tri
    consts[:, 256:384] = tri
    consts[:, 384:416] = 1.0

    in_maps = []
    for c in range(NCORES):
        b, g = divmod(c, 4)
        heads = [4 * g + i for i in range(4)]
        # wq_local: per unit u: [q(128) | k(128) | v(128)] for heads
        # (4g+2u, 4g+2u+1)
        cols = []
        for u in range(2):
            h0, h1 = heads[2 * u], heads[2 * u + 1]
            for part in range(3):  # q, k, v sections at offsets 0, D, 2D
                off = part * D
                cols.append(w_qkv[:, off + h0 * DH: off + (h0 + 1) * DH])
                cols.append(w_qkv[:, off + h1 * DH: off + (h1 + 1) * DH])
        wql = np.concatenate(cols, axis=1)          # [1024, 768]
        # section-major SBUF layout: [128 p, 6 sections, 8 d, 128]
        wq_local = np.ascontiguousarray(
            wql.reshape(8, 128, 6, 128).transpose(1, 2, 0, 3)
            .reshape(128, 6 * 8 * 128)).astype(np.float16)
        # wout_local[u]: rows for heads (4g+2u, 4g+2u+1) stacked [64+64, 1024]
        wo = np.zeros((2, 128, 1024), dtype=np.float16)
        for u in range(2):
            h0, h1 = heads[2 * u], heads[2 * u + 1]
            wo[u, 0:64] = w_out[h0 * DH:(h0 + 1) * DH, :]
            wo[u, 64:128] = w_out[h1 * DH:(h1 + 1) * DH, :]
        in_maps.append({
            "xt": xts[b],
            "wq": wq_local,
            "wout": wo,
            "consts": consts,
        })
    return in_maps


def kernel(x, w_qkv, w_out):
    global LAST_RESULT
    if "nc" not in _CACHE:
        _CACHE["nc"] = _build()
    nc = _CACHE["nc"]
    in_maps = _host_inputs(x, w_qkv, w_out)
    res = run_bass_kernel_spmd(nc, in_maps, list(range(NCORES)))
    LAST_RESULT = res
    y = np.zeros((B, L, D), dtype=np.float32)
    for c in range(NCORES):
        y[c // 4] += res.results[c]["y"].astype(np.float32)
    return y


# revision 75
# speedup vs baseline: 1.0195x; 1.0195x over previous
"""Causal self-attention TRN2 kernel (B=2, L=2048, D=1024, H=16, dh=64).

Sharding: 8 cores = 2 batches x 4 head-groups. Core c handles batch c//4 and
heads [4g..4g+3] where g = c % 4, as two head-pairs ("units").

Per-core device program (SPMD, same program all cores, different data),
everything fp16 on the PE (full rate) with fp32 PSUM accumulation:
  phase 1 (per unit): qT/kT = W.T @ x (head dims on partitions); V computed
      directly in [kpos, dh] orientation with a fused ones column (v65) so
      the AV matmul also produces softmax row-sums.
  phase 2 (per unit, per 512-wide q-block): S^T chunks [128k x 512q] on PE,
      exp on ScalarE (PSUM->SBUF fp16, scale=1/8, no max-sub), causal
      triangle mask via one fp16 DVE multiply per diagonal chunk, then
      flipped AV: the P^T chunk is the stationary operand and v65 the
      moving one -> [128q, 64dh | l] PSUM tiles. Normalize with
      per-partition scalars (DVE reciprocal + tensor_scalar_mul),
      PE-transpose back into outT. AV consumption runs DEPTH chunks behind
      S production so the PE never waits on the ScalarE exp; each block's
      normalize/transpose runs as the next block's prelude.
  phase 3: y[q,:] partial = outT_u0/u1 x w_out, fp32 PSUM, fp16 DMA out.
Host: y[b] = sum of the 4 partial outputs for batch b.
PE work is interleaved across phases (qkv(u1) inside the attn(u0) window,
outproj inside the attn(u1) window) to keep the tensor engine saturated.
"""

import numpy as np

import concourse.bass as bass
import concourse.mybir as mybir
from concourse import bacc
import concourse.tile as tile
from concourse.bass_utils import run_bass_kernel_spmd

F32 = mybir.dt.float32
FP16 = mybir.dt.float16
EXP = mybir.ActivationFunctionType.Exp
COPY = mybir.ActivationFunctionType.Copy

B, L, D = 2, 2048, 1024
H, DH = 16, 64
NCORES = 8
NQB = L // 512          # q-blocks per sequence (4)
DEPTH = 5               # chunks of S/exp lookahead before AV consumes

_CACHE = {}
LAST_RESULT = None      # BassKernelResults of the most recent run (for test.py)
DEBUG_TAPS = False      # adds qT/kT/v65/outT dram outputs for debug.py


def _build():
    nc = bacc.Bacc("TRN2", target_bir_lowering=False, debug=False,
                   num_devices=NCORES)

    xt_d = nc.dram_tensor("xt", [D, L], FP16, kind="ExternalInput").ap()
    # wq pre-packed on host to the SBUF layout: [128, 6 sections, 8 d, 128]
    wq_d = nc.dram_tensor("wq", [128, 6 * 8 * 128], FP16,
                          kind="ExternalInput").ap()
    wout_d = nc.dram_tensor("wout", [2, 128, 1024], FP16,
                            kind="ExternalInput").ap()
    consts_d = nc.dram_tensor("consts", [128, 416], FP16,
                              kind="ExternalInput").ap()
    y_d = nc.dram_tensor("y", [L, D], FP16, kind="ExternalOutput").ap()

    with tile.TileContext(nc) as tc:
        with tc.tile_pool(name="persist", bufs=1) as pp, \
             tc.tile_pool(name="work", bufs=1) as wp, \
             tc.tile_pool(name="psstage", bufs=3, space="PSUM") as ps_stage, \
             tc.tile_pool(name="psav", bufs=2, space="PSUM") as ps_av:

            # ---- input DMAs (ordered so early chains' deps land first) ----
            # wq SBUF layout is section-major: section s = (u*3 + ci), cols
            # [s*1024 + d*128 + c]; each section is one contiguous 728ns DMA.
            wq = pp.tile([128, 6 * 1024], FP16, name="wq")
            xt = pp.tile([128, 8 * L], FP16, name="xt")
            xt_dst = xt.rearrange("p (d l) -> p d l", d=8)
            xt_src = xt_d.rearrange("(a b) c -> b a c", b=128)

            def load_wq(u, ci):
                s = u * 3 + ci
                nc.sync.dma_start(out=wq[:, s * 1024:(s + 1) * 1024],
                                  in_=wq_d[:, s * 1024:(s + 1) * 1024])

            def load_xt(q, half=None, eng=None):
                cs = slice(q * 512, (q + 1) * 512) if half is None else \
                    slice(q * 512 + half * 256, q * 512 + half * 256 + 256)
                (eng or nc.sync).dma_start(out=xt_dst[:, :, cs],
                                           in_=xt_src[:, :, cs])

            load_wq(0, 0)
            load_xt(0, 0)
            load_xt(0, 1)
            load_wq(0, 1)
            load_wq(0, 2)
            consts = pp.tile([128, 416], FP16, name="consts")
            nc.sync.dma_start(out=consts, in_=consts_d)
            ident = consts[:, 0:128]      # eye(128)
            tri2 = consts[:, 128:384].rearrange("p (h q) -> p h q", h=2)
            ones32 = consts[:, 384:416]
            for ci in range(3):
                load_wq(1, ci)
            load_xt(1)
            load_xt(2)
            load_xt(3)
            wout = pp.tile([128, 2048], FP16, name="wout")
            nc.sync.dma_start(out=wout,
                              in_=wout_d.rearrange("a b c -> b a c"))

            # persistent per-unit tensors
            qT = [pp.tile([128, L], FP16, name=f"qT{u}") for u in range(2)]
            kT = [pp.tile([128, L], FP16, name=f"kT{u}") for u in range(2)]
            # V with ones column: per head 16 chunks x (64 dh + 1)
            v65 = [pp.tile([128, 2 * 16 * 65], FP16, name=f"v65{u}")
                   for u in range(2)]
            outT = [pp.tile([128, L], FP16, name=f"outT{u}") for u in range(2)]

            def copy_op(act, out, in_):
                # Act only while it is otherwise idle (window A); fillers
                # inside the attn windows copy on DVE to keep exp flowing.
                if act:
                    nc.scalar.activation(out, in_, COPY)
                else:
                    nc.vector.tensor_copy(out, in_)

            def emit_qk_half(u, ci, r, w=512, act=False):
                """One 512-col half: 8 accumulating matmuls + one copy."""
                dest = qT[u] if ci == 0 else kT[u]
                so = (u * 3 + ci) * 1024
                acc = ps_stage.tile([128, 512], F32,
                                    name=f"qk{u}{ci}{r}", tag="stage")
                for piece in range(512 // w):
                    c0 = r * 512 + piece * w
                    for d in range(8):
                        nc.tensor.matmul(
                            acc[:, piece * w:(piece + 1) * w],
                            wq[:, so + d * 128:so + (d + 1) * 128],
                            xt[:, d * L + c0:d * L + c0 + w],
                            start=(d == 0), stop=(d == 7),
                            skip_group_check=True)
                copy_op(act, dest[:, r * 512:(r + 1) * 512], acc)

            def emit_v_chain(u, g, act=False):
                # 4 k-chunks of 128; V lands in [kpos, dh] orientation
                so = (u * 3 + 2) * 1024
                acc = ps_stage.tile([128, 512], F32, name=f"v{u}{g}",
                                    tag="stage")
                for jc in range(4):
                    j = g * 4 + jc
                    for d in range(8):
                        nc.tensor.matmul(
                            acc[:, jc * 128:(jc + 1) * 128],
                            xt[:, d * L + j * 128:d * L + (j + 1) * 128],
                            wq[:, so + d * 128:so + (d + 1) * 128],
                            start=(d == 0), stop=(d == 7),
                            skip_group_check=True)
                accv = acc.rearrange("p (c v) -> p c v", c=4)
                v65v = v65[u].rearrange("p (c v) -> p c v", c=32)
                for h in range(2):
                    copy_op(act, v65v[:, h * 16 + g * 4:h * 16 + (g + 1) * 4,
                                      0:64],
                            accv[:, :, h * 64:(h + 1) * 64])

            def qkv_chains(u, act3=False):
                # half-granularity (~1.7us each) so fillers never starve Act;
                # ordered so the first chains only need xt q-blocks 0/1
                w0 = 256 if u == 0 else 512
                yield lambda: emit_qk_half(u, 0, 0, w=w0, act=act3)
                yield lambda: emit_qk_half(u, 0, 1, w=w0, act=act3)
                yield lambda: emit_qk_half(u, 1, 0, act=act3)
                yield lambda: emit_qk_half(u, 1, 1, act=act3)
                yield lambda: emit_v_chain(u, 0)
                yield lambda: emit_v_chain(u, 1)
                yield lambda: emit_qk_half(u, 0, 2)
                yield lambda: emit_qk_half(u, 0, 3)
                yield lambda: emit_qk_half(u, 1, 2)
                yield lambda: emit_qk_half(u, 1, 3)
                yield lambda: emit_v_chain(u, 2)
                yield lambda: emit_v_chain(u, 3)

            def finalize_norm(u, I, h, av):
                """DVE part: 1/l and normalize the 4 q-tiles of head h."""
                avv = av.rearrange("p (t c) -> p t c", t=4)
                rt = wp.tile([128, 4], F32, name=f"rt{u}{I}{h}",
                             tag="rt", bufs=4)
                rtv = rt.rearrange("p (t c) -> p t c", c=1)
                nc.vector.reciprocal(rtv, avv[:, :, 64:65])
                osb = wp.tile([128, 256], FP16, name=f"osb{u}{I}{h}",
                              tag="osb", bufs=4)
                osbv = osb.rearrange("p (t c) -> p t c", t=4)
                nc.vector.tensor_mul(osbv, avv[:, :, 0:64],
                                     rtv.broadcast_to([128, 4, 64]))
                return osb

            def finalize_tp(u, I, h, osb):
                """PE part: transpose normalized tiles into outT."""
                tp = ps_stage.tile([128, 512], FP16, name=f"tp{u}{I}{h}",
                                   tag="stage")
                for t in range(4):
                    nc.tensor.transpose(tp[0:64, t * 128:(t + 1) * 128],
                                        osb[:, t * 64:(t + 1) * 64], ident)
                nc.vector.tensor_copy(
                    outT[u][h * 64:(h + 1) * 64, I * 512:(I + 1) * 512],
                    tp[0:64, :])

            def finalize_qt_norm(u, I, h, t, av):
                """Tail variant, DVE part for a single q-tile."""
                avv = av.rearrange("p (t c) -> p t c", t=4)
                rt = wp.tile([128, 4], F32, name=f"rt{u}{I}{h}{t}",
                             tag="rt", bufs=4)
                nc.vector.reciprocal(rt[:, 0:1], avv[:, t:t + 1, 64:65])
                osb = wp.tile([128, 256], FP16, name=f"ot{u}{I}{h}{t}",
                              tag="osb", bufs=4)
                nc.vector.tensor_scalar_mul(osb[:, 0:64],
                                            avv[:, t:t + 1, 0:64],
                                            rt[:, 0:1])
                return osb

            def finalize_qt_tp(u, I, h, t, osb):
                """Tail variant, PE part (stage-tag psum: av stays live)."""
                tp = ps_stage.tile([128, 128], FP16, name=f"tq{u}{I}{h}{t}",
                                   tag="stage")
                nc.tensor.transpose(tp[0:64, :], osb[:, 0:64], ident)
                nc.vector.tensor_copy(
                    outT[u][h * 64:(h + 1) * 64,
                            (4 * I + t) * 128:(4 * I + t + 1) * 128],
                    tp[0:64, :])

            def outproj_qc(qc, act_copy=False, split=False):
                qs = slice(qc * 128, (qc + 1) * 128)
                ysb = wp.tile([128, 1024], FP16, name=f"ys{qc}",
                              tag="ysb", bufs=4)
                for nck in range(2):
                    ns = slice(nck * 512, (nck + 1) * 512)
                    yps = ps_stage.tile([128, 512], F32,
                                        name=f"y{qc}{nck}", tag="stage")
                    nc.tensor.matmul(yps, outT[0][:, qs], wout[:, ns],
                                     start=True, stop=False,
                                     skip_group_check=True)
                    nc.tensor.matmul(yps, outT[1][:, qs],
                                     wout[:, 1024 + ns.start:1024 + ns.stop],
                                     start=False, stop=True,
                                     skip_group_check=True)
                    if act_copy and nck == 0:
                        nc.scalar.activation(ysb[:, ns], yps, COPY)
                    else:
                        nc.vector.tensor_copy(ysb[:, ns], yps)
                    if split:   # last q-tile: stream each half immediately
                        nc.sync.dma_start(out=y_d[qs, ns], in_=ysb[:, ns])
                if not split:
                    nc.sync.dma_start(out=y_d[qs, :], in_=ysb)

            def attn_block(u, I, prelude, chainq, oprojq, fill_phase,
                           fill_every, fill_max, tail=False, depth=DEPTH):
                nj = 4 * (I + 1)
                av = [None, None]
                pts = []
                pops = 0
                opops = 0
                osbs = {}        # tail: (h, t) -> osb awaiting transpose
                for j in range(nj + depth + (3 if tail else 0)):
                    if j < nj:
                        m = j - 4 * I
                        qlo = max(m, 0) * 128
                        st = ps_stage.tile([128, 1024], F32,
                                           name=f"st{u}{I}{j}", tag="stage")
                        for h in range(2):
                            nc.tensor.matmul(
                                st[:, h * 512 + qlo:(h + 1) * 512],
                                kT[u][h * 64:(h + 1) * 64,
                                      j * 128:(j + 1) * 128],
                                qT[u][h * 64:(h + 1) * 64,
                                      I * 512 + qlo:(I + 1) * 512],
                                start=True, stop=True, skip_group_check=True)
                        pt = wp.tile([128, 1024], FP16, name=f"pt{u}{I}{j}",
                                     tag="pt", bufs=9)
                        stv = st.rearrange("p (h q) -> p h q", h=2)
                        ptv = pt.rearrange("p (h q) -> p h q", h=2)
                        nc.scalar.activation(ptv[:, :, qlo:512],
                                             stv[:, :, qlo:512], EXP,
                                             scale=0.125)
                        pts.append(pt)
                    # masks trail S/exp by 2 chunks so DVE never queues on Act
                    mc = j - 2
                    if 0 <= mc < nj and mc >= 4 * I:
                        mqlo = (mc - 4 * I) * 128
                        pmv = pts[mc].rearrange("p (h q) -> p h q", h=2)
                        nc.gpsimd.tensor_mul(pmv[:, :, mqlo:mqlo + 128],
                                             pmv[:, :, mqlo:mqlo + 128],
                                             tri2)
                    if j == 2:
                        # av groups share one psum bank, so never start=True
                        # (start lazily zeroes the whole 2KB region, wiping
                        # sibling q-tile accumulators): memset instead.
                        for h in range(2):
                            av[h] = ps_av.tile([128, 260], F32,
                                               name=f"av{u}{I}{h}", tag="av")
                            nc.vector.memset(av[h], 0.0)
                    if j == 3:
                        for f in prelude:      # prev block transposes (PE)
                            f()
                    if j >= fill_phase and (j - fill_phase) % fill_every == 0:
                        if chainq and pops < fill_max[0]:
                            chainq.pop(0)()
                            pops += 1
                        elif oprojq and opops < fill_max[1]:
                            oprojq.pop(0)()
                            opops += 1
                    ja = j - depth
                    if 0 <= ja < nj:
                        ma = ja - 4 * I
                        pta = pts[ja]
                        for t in range(max(ma, 0), 4):
                            qt = 4 * I + t
                            for h in range(2):
                                nc.tensor.matmul(
                                    av[h][:, t * 65:t * 65 + 65],
                                    pta[:, h * 512 + t * 128:
                                        h * 512 + (t + 1) * 128],
                                    v65[u][:, h * 1040 + ja * 65:
                                           h * 1040 + (ja + 1) * 65],
                                    start=False, stop=(ja == qt),
                                    skip_group_check=True)
                        if tail and ma >= 0:
                            for h in range(2):
                                osbs[(h, ma)] = finalize_qt_norm(
                                    u, I, h, ma, av[h])
                    if tail:
                        mb = j - depth - 1 - 4 * I
                        if mb >= 0 and (0, mb) in osbs:
                            for h in range(2):
                                finalize_qt_tp(u, I, h, mb,
                                               osbs.pop((h, mb)))
                        m2 = j - depth - 2 - 4 * I
                        if 0 <= m2 <= 3:
                            outproj_qc(4 * I + m2, act_copy=True,
                                       split=(m2 == 3))
                if tail:
                    return []
                # norms (DVE) run now, right after the final AV stops; the
                # PE transposes become the next block's prelude.
                osbf = [finalize_norm(u, I, h, av[h]) for h in range(2)]
                return [lambda h=h, o=osbf[h]: finalize_tp(u, I, h, o)
                        for h in range(2)]

            # ---- emission schedule ----
            # warm the Act exp table while the PE is still loading inputs
            junk = wp.tile([128, 1], FP16, name="junk", tag="junk", bufs=1)
            nc.vector.memset(junk, 0.0)
            nc.scalar.activation(junk, junk, EXP, scale=0.0)
            for u in range(2):
                nc.vector.tensor_copy(
                    v65[u].rearrange("p (c v) -> p c v", c=32)[:, :, 64:65],
                    ones32)
            c0 = list(qkv_chains(0))
            for ch in [c0[0], c0[2], c0[4]]:  # Qh0, Kh0, Vg0: block 0's deps
                ch()
            c1 = list(qkv_chains(1))
            # u0/u1 blocks alternate; two filler queues ordered by first use
            # (and by xt q-block arrival) so outproj lands in the exp-heavy
            # late blocks.
            chainq = [c1[0], c1[2], c1[4],          # u1 Qh0, Kh0, Vg0
                      c0[1], c0[3], c0[5],          # u0 Qh1, Kh1, Vg1
                      c1[1], c1[3], c1[5],          # u1 Qh1, Kh1, Vg1
                      c0[6], c0[8],                 # u0 Qh2, Kh2
                      c1[6], c1[8],                 # u1 Qh2, Kh2
                      c0[10], c1[10],               # Vg2 u0, u1
                      c0[7], c0[9], c0[11],         # u0 Qh3, Kh3, Vg3
                      c1[7], c1[9], c1[11]]         # u1 Qh3, Kh3, Vg3
            oprojq = []
            prel = []
            sched = [  # (u, I, phase, every, (chain_max, oproj_max), depth)
                (0, 0, 2, 1, (3, 0), 5), (1, 0, 3, 1, (3, 0), 5),
                (0, 1, 2, 2, (3, 0), 5), (1, 1, 4, 2, (3, 1), 5),
                (0, 2, 2, 2, (2, 1), 5), (1, 2, 4, 2, (2, 2), 5),
                (0, 3, 2, 2, (4, 3), 5), (1, 3, 4, 2, (1, 99), 5)]
            for u, I, phase, every, fmax, dep in sched:
                tail = (u == 1 and I == NQB - 1)
                prel = attn_block(u, I, prel, chainq, oprojq,
                                  fill_phase=phase, fill_every=every,
                                  fill_max=fmax, tail=tail, depth=dep)
                if u == 1 and I < NQB - 1:
                    oprojq += [lambda qc=qc: outproj_qc(qc)
                               for qc in range(4 * I, 4 * (I + 1))]
            while chainq:
                chainq.pop(0)()
            while oprojq:
                oprojq.pop(0)()

            if DEBUG_TAPS:
                taps = {"qT0": qT[0], "kT0": kT[0], "outT0": outT[0],
                        "v650": v65[0]}
                for nm, t_sb in taps.items():
                    td = nc.dram_tensor(nm, list(t_sb.shape), FP16,
                                        kind="ExternalOutput").ap()
                    nc.sync.dma_start(out=td, in_=t_sb)

    nc.compile()
    return nc


def _host_inputs(x, w_qkv, w_out):
    """Build per-core input maps."""
    x = np.asarray(x, dtype=np.float32)
    w_qkv = np.asarray(w_qkv, dtype=np.float32)
    w_out = np.asarray(w_out, dtype=np.float32)

    xts = [np.ascontiguousarray(x[b].T).astype(np.float16) for b in range(B)]

    consts = np.zeros((128, 416), dtype=np.float16)
    consts[:, 0:128] = np.eye(128)
    kk = np.arange(128)[:, None]
    qq = np.arange(128)[None, :]
    tri = (kk <= qq).astype(np.float16)
    consts[:, 128:256] = tri
    consts[:, 256:384] = tri
    consts[:, 384:416] = 1.0

    in_maps = []
    for c in range(NCORES):
        b, g = divmod(c, 4)
        heads = [4 * g + i for i in range(4)]
        # wq_local: per unit u: [q(128) | k(128) | v(128)] for heads
        # (4g+2u, 4g+2u+1)
        cols = []
        for u in range(2):
            h0, h1 = heads[2 * u], heads[2 * u + 1]
            for part in range(3):  # q, k, v sections at offsets 0, D, 2D
                off = part * D
                cols.append(w_qkv[:, off + h0 * DH: off + (h0 + 1) * DH])
                cols.append(w_qkv[:, off + h1 * DH: off + (h1 + 1) * DH])
        wql = np.concatenate(cols, axis=1)          # [1024, 768]
        # section-major SBUF layout: [128 p, 6 sections, 8 d, 128]
        wq_local = np.ascontiguousarray(
            wql.reshape(8, 128, 6, 128).transpose(1, 2, 0, 3)
            .reshape(128, 6 * 8 * 128)).astype(np.float16)
        # wout_local[u]: rows for heads (4g+2u, 4g+2u+1) stacked [64+64, 1024]
        wo = np.zeros((2, 128, 1024), dtype=np.float16)
        for u in range(2):
            h0, h1 = heads[2 * u], heads[2 * u + 1]
            wo[u, 0:64] = w_out[h0 * DH:(h0 + 1) * DH, :]
            wo[u, 64:128] = w_out[h1 * DH:(h1 + 1) * DH, :]
        in_maps.append({
            "xt": xts[b],
            "wq": wq_local,
            "wout": wo,
            "consts": consts,
        })
    return in_maps


def kernel(x, w_qkv, w_out):
    global LAST_RESULT
    if "nc" not in _CACHE:
        _CACHE["nc"] = _build()
    nc = _CACHE["nc"]
    in_maps = _host_inputs(x, w_qkv, w_out)
    res = run_bass_kernel_spmd(nc, in_maps, list(range(NCORES)))
    LAST_RESULT = res
    y = np.zeros((B, L, D), dtype=np.float32)
    for c in range(NCORES):
        y[c // 4] += res.results[c]["y"].astype(np.float32)
    return y
